# revision 14
# baseline (speedup 1.0000x reference)
"""Multi-head attention (N=2, L=2048, E=1024, H=16) on 8 TRN2 NeuronCores.

Sharding: each core owns one batch (core//4) and a 512-query slice
(core%4).  It computes K/V projections for its whole batch (replicated
4x across the cores sharing that batch), Q only for its query slice,
full softmax attention for its queries, and the output projection for
its slice.  Output shards are disjoint, so the host just concatenates —
no collectives (an on-chip 16MB AllReduce would cost ~300us, far more
than the replicated K/V matmuls).

All matmuls run in bf16 with fp32 PSUM accumulation.  The 1/sqrt(E)
score scale is folded into Wq on the host.  Softmax skips the max
subtraction (scores are ~N(0, 0.25^2) by construction — no overflow
risk) and gets the row sums for free by augmenting V with a ones
column, so the only non-matmul softmax cost is the exp itself (ACT).

Layouts on device (per core):
  xT   [e, l]   : x[n].T          — rhs for K^T, lhsT for V
  xqT  [e, q]   : x[n, qs:qs+512].T
  w*T  [e, eo]  : W.T             — lhsT for the projections
  K^T  [eo, l]  (eo = 64*h + d)   — head h lives at partition block
  Q^T  [eo, q]                      h//2, offset 64*(h%2), giving free
                                    row-group pairing of the d=64
                                    scores matmuls for head pairs.
  V    [l, h, 65] (col 64 = ones) — lhsT for ctx^T; row 64 of the ctx
                                    PSUM then holds the softmax sums.
  scores^T [k, q] -> exp -> p^T   — ctx^T[d, q] = V'.T @ p^T
  ctxN [eo, q] = ctx^T * (1/sums) — lhsT for the output projection.
"""

import os
import sys
from contextlib import ExitStack

import numpy as np

if "/opt/trn_rl_repo" not in sys.path:
    sys.path.insert(0, "/opt/trn_rl_repo")

import ml_dtypes

import concourse.bass as bass
import concourse.mybir as mybir
import concourse.tile as tile
from concourse import bacc
from concourse.bass_utils import run_bass_kernel_spmd

EMBED = 1024
HEADS = 16
DHEAD = 64
N_BATCH = 2
L = 2048
LQ = 512          # queries per core
EB = 8            # 128-row blocks of the embed dim
LB = 16           # 128-row blocks of the key dim
P = 128
NCORES = 8

BF16 = mybir.dt.bfloat16
F32 = mybir.dt.float32


def _build_bass(debug=False):
    nc = bacc.Bacc()

    xT = nc.dram_tensor("xT", (EB, P, L), BF16, kind="ExternalInput")
    xqT = nc.dram_tensor("xqT", (EB, P, LQ), BF16, kind="ExternalInput")
    wqT = nc.dram_tensor("wqT", (EB, P, EMBED), BF16, kind="ExternalInput")
    wkT = nc.dram_tensor("wkT", (EB, P, EMBED), BF16, kind="ExternalInput")
    wvT = nc.dram_tensor("wvT", (EB, P, EMBED), BF16, kind="ExternalInput")
    woT = nc.dram_tensor("woT", (EB, P, EMBED), BF16, kind="ExternalInput")
    bo = nc.dram_tensor("bo", (1, EMBED), BF16, kind="ExternalInput")
    out = nc.dram_tensor("out", (LQ // P, P, EMBED), F32, kind="ExternalOutput")

    dbg = None
    if debug:
        dbg = {
            "dbg_QT": nc.dram_tensor("dbg_QT", (EB, P, LQ), BF16, kind="ExternalOutput"),
            "dbg_KT": nc.dram_tensor("dbg_KT", (EB, P, L), BF16, kind="ExternalOutput"),
            "dbg_V": nc.dram_tensor(
                "dbg_V", (LB, P, HEADS * (DHEAD + 1)), BF16, kind="ExternalOutput"
            ),
            "dbg_pt": nc.dram_tensor("dbg_pt", (4, P, 2048), BF16, kind="ExternalOutput"),
            "dbg_cN": nc.dram_tensor("dbg_cN", (EB, P, LQ), BF16, kind="ExternalOutput"),
            "dbg_ctx": nc.dram_tensor("dbg_ctx", (2, P, LQ), F32, kind="ExternalOutput"),
            "dbg_recip": nc.dram_tensor("dbg_recip", (2, 1, LQ), F32, kind="ExternalOutput"),
            "dbg_bcs": nc.dram_tensor("dbg_bcs", (2, 64, LQ), F32, kind="ExternalOutput"),
        }

    with tile.TileContext(nc) as tc, ExitStack() as ctx:
        _body(nc, tc, ctx, xT, xqT, wqT, wkT, wvT, woT, bo, out, dbg)
    nc.compile()
    return nc


def _body(nc, tc, ctx, xT, xqT, wqT, wkT, wvT, woT, bo, out, dbg=None):
    Exp = mybir.ActivationFunctionType.Exp

    persist = ctx.enter_context(tc.tile_pool(name="persist", bufs=1))

    def load(pool, dram_ap, shape, tag):
        t = pool.tile(shape, BF16, tag=tag, name=tag)
        nc.sync.dma_start(out=t, in_=dram_ap)
        return t

    # ---- persistent inputs -------------------------------------------------
    wo_sb = [load(persist, woT[i], [P, EMBED], f"wo{i}") for i in range(EB)]
    bo_sb = load(persist, bo.ap(), [1, EMBED], "bo")
    ones32 = persist.tile([1, P], F32, tag="ones32", name="ones32")
    nc.vector.memset(ones32, 1.0)
    ones16 = persist.tile([1, P], BF16, tag="ones16", name="ones16")
    nc.vector.memset(ones16, 1.0)

    # ---- persistent intermediates ------------------------------------------
    KT_sb = [persist.tile([P, L], BF16, tag=f"KT{i}", name=f"KT{i}") for i in range(EB)]
    QT_sb = [persist.tile([P, LQ], BF16, tag=f"QT{i}", name=f"QT{i}") for i in range(EB)]
    V_sb = [
        persist.tile([P, HEADS, DHEAD + 1], BF16, tag=f"V{i}", name=f"V{i}")
        for i in range(LB)
    ]
    cN_sb = [persist.tile([P, LQ], BF16, tag=f"cN{i}", name=f"cN{i}") for i in range(EB)]

    # ---- phases A/B: projections; input pools freed afterwards -------------
    with tc.tile_pool(name="poolB", bufs=1) as poolB:
        xT_sb = [load(poolB, xT[i], [P, L], f"xT{i}") for i in range(EB)]
        wv_sb = [load(poolB, wvT[i], [P, EMBED], f"wv{i}") for i in range(EB)]

        # ---- phase A: Q^T and K^T projections ------------------------------
        with (
            tc.tile_pool(name="poolA", bufs=1) as poolA,
            tc.tile_pool(name="psA", bufs=2, space="PSUM") as psA,
        ):
            xq_sb = [load(poolA, xqT[i], [P, LQ], f"xq{i}") for i in range(EB)]
            wq_sb = [load(poolA, wqT[i], [P, EMBED], f"wq{i}") for i in range(EB)]
            wk_sb = [load(poolA, wkT[i], [P, EMBED], f"wk{i}") for i in range(EB)]

            for eo in range(EB):
                psq = psA.tile([P, LQ], F32, tag="psA", name="psq")
                for e in range(EB):
                    nc.tensor.matmul(
                        psq,
                        wq_sb[e][:, eo * P : (eo + 1) * P],
                        xq_sb[e],
                        start=(e == 0),
                        stop=(e == EB - 1),
                    )
                nc.vector.tensor_copy(out=QT_sb[eo], in_=psq)
            for eo in range(EB):
                psk = psA.tile([P, L], F32, tag="psA", name="psk")
                for e in range(EB):
                    lhsT = wk_sb[e][:, eo * P : (eo + 1) * P]
                    for c in range(L // 512):
                        nc.tensor.matmul(
                            psk[:, c * 512 : (c + 1) * 512],
                            lhsT,
                            xT_sb[e][:, c * 512 : (c + 1) * 512],
                            start=(e == 0),
                            stop=(e == EB - 1),
                        )
                nc.vector.tensor_copy(out=KT_sb[eo], in_=psk)

        # ---- phase B: V projection (with ones column) ----------------------
        with tc.tile_pool(name="psB", bufs=2, space="PSUM") as psB:
            for lb in range(LB):
                psv = psB.tile([P, EMBED], F32, tag="psV", name="psv")
                for e in range(EB):
                    lhsT = xT_sb[e][:, lb * P : (lb + 1) * P]
                    for c in range(EMBED // 512):
                        nc.tensor.matmul(
                            psv[:, c * 512 : (c + 1) * 512],
                            lhsT,
                            wv_sb[e][:, c * 512 : (c + 1) * 512],
                            start=(e == 0),
                            stop=(e == EB - 1),
                        )
                nc.vector.memset(V_sb[lb][:, :, DHEAD : DHEAD + 1], 1.0)
                nc.vector.tensor_copy(
                    out=V_sb[lb][:, :, 0:DHEAD],
                    in_=psv.rearrange("p (h d) -> p h d", d=DHEAD),
                )

    # ---- phase C: attention, one head pair at a time -----------------------
    with (
        tc.tile_pool(name="psS", bufs=1, space="PSUM") as psS,
        tc.tile_pool(name="psCtx", bufs=2, space="PSUM") as psCtx,
        tc.tile_pool(name="psBc", bufs=1, space="PSUM") as psBc,
        tc.tile_pool(name="ptp", bufs=10) as ptp,
        tc.tile_pool(name="smp", bufs=4) as smp,
    ):
        # slab s of each score/p tile: s0=(hA, 2g), s1=(hB, 2g),
        # s2=(hA, 2g+1), s3=(hB, 2g+1) — A/B alternate PE row groups.
        def slab(s, g):
            hi = s % 2
            kb = 2 * g + s // 2
            return hi, kb

        for j in range(HEADS // 2):
            pts = []
            for g in range(8):
                pss = psS.tile([P, 2048], F32, tag="ps_s", name="pss")
                for s in range(4):
                    hi, kb = slab(s, g)
                    off = 64 * hi
                    nc.tensor.matmul(
                        pss[:, s * 512 : (s + 1) * 512],
                        KT_sb[j][off : off + 64, kb * P : (kb + 1) * P],
                        QT_sb[j][off : off + 64, :],
                        start=True,
                        stop=True,
                    )
                pt = ptp.tile([P, 2048], BF16, tag="pt", name="pt")
                nc.scalar.activation(out=pt, in_=pss, func=Exp)
                if dbg is not None and j == 0 and g < 4:
                    nc.sync.dma_start(out=dbg["dbg_pt"][g], in_=pt)
                pts.append(pt)
            cps = [
                psCtx.tile([P, LQ], F32, tag="ctx", name="cpsA"),
                psCtx.tile([P, LQ], F32, tag="ctx", name="cpsB"),
            ]
            for g in range(8):
                for s in range(4):
                    hi, kb = slab(s, g)
                    nc.tensor.matmul(
                        cps[hi][0 : DHEAD + 1, :],
                        V_sb[kb][:, 2 * j + hi, :],
                        pts[g][:, s * 512 : (s + 1) * 512],
                        start=(g == 0 and s < 2),
                        stop=(g == 7 and s >= 2),
                    )
            for hi in range(2):
                if dbg is not None and j == 0:
                    cdump = smp.tile([P, LQ], F32, tag="cdump", name="cdump")
                    nc.vector.tensor_copy(out=cdump, in_=cps[hi])
                    nc.sync.dma_start(out=dbg["dbg_ctx"][hi], in_=cdump)
                recip = smp.tile([1, LQ], F32, tag="recip", name="recip")
                nc.vector.reciprocal(out=recip, in_=cps[hi][DHEAD : DHEAD + 1, :])
                bc = psBc.tile([64, LQ], F32, tag="bc", name="bc")
                nc.tensor.matmul(bc, ones32[:, 0:64], recip, start=True, stop=True)
                bcs = smp.tile([64, LQ], F32, tag="bcs", name="bcs")
                nc.vector.tensor_copy(out=bcs, in_=bc)
                if dbg is not None and j == 0:
                    nc.sync.dma_start(out=dbg["dbg_recip"][hi], in_=recip)
                    nc.sync.dma_start(out=dbg["dbg_bcs"][hi], in_=bcs)
                nc.vector.tensor_mul(
                    cN_sb[j][64 * hi : 64 * hi + 64, :],
                    cps[hi][0:DHEAD, :],
                    bcs,
                )

    # ---- phase D: output projection + bias ---------------------------------
    with (
        tc.tile_pool(name="psD", bufs=2, space="PSUM") as psD,
        tc.tile_pool(name="osb", bufs=2) as osb,
    ):
        for qb in range(LQ // P):
            pso = psD.tile([P, EMBED], F32, tag="po", name="pso")
            for eb in range(EB):
                lhsT = cN_sb[eb][:, qb * P : (qb + 1) * P]
                for c in range(EMBED // 512):
                    nc.tensor.matmul(
                        pso[:, c * 512 : (c + 1) * 512],
                        lhsT,
                        wo_sb[eb][:, c * 512 : (c + 1) * 512],
                        start=(eb == 0),
                        stop=False,
                    )
            for c in range(EMBED // 512):
                nc.tensor.matmul(
                    pso[:, c * 512 : (c + 1) * 512],
                    ones16[:, 0:P],
                    bo_sb[:, c * 512 : (c + 1) * 512],
                    start=False,
                    stop=True,
                )
            ot = osb.tile([P, EMBED], F32, tag="ot", name="ot")
            nc.vector.tensor_copy(out=ot, in_=pso)
            nc.sync.dma_start(out=out[qb], in_=ot)

    if dbg is not None:
        for i in range(EB):
            nc.sync.dma_start(out=dbg["dbg_QT"][i], in_=QT_sb[i])
            nc.sync.dma_start(out=dbg["dbg_KT"][i], in_=KT_sb[i])
            nc.sync.dma_start(out=dbg["dbg_cN"][i], in_=cN_sb[i])
        for i in range(LB):
            nc.sync.dma_start(
                out=dbg["dbg_V"][i],
                in_=V_sb[i].rearrange("p h d -> p (h d)"),
            )


_NC_CACHE = None


def _get_nc():
    global _NC_CACHE
    if _NC_CACHE is None:
        _NC_CACHE = _build_bass()
    return _NC_CACHE


def _make_in_maps(x, Wq, Wk, Wv, Wo, bo):
    bf = ml_dtypes.bfloat16
    xb = np.asarray(x, dtype=np.float32).astype(bf)
    scale = 1.0 / np.sqrt(np.float32(EMBED))
    wqTb = np.ascontiguousarray(np.asarray(Wq, np.float32).T * scale).astype(bf)
    wkTb = np.ascontiguousarray(np.asarray(Wk, np.float32).T).astype(bf)
    wvTb = np.ascontiguousarray(np.asarray(Wv, np.float32).T).astype(bf)
    woTb = np.ascontiguousarray(np.asarray(Wo, np.float32).T).astype(bf)
    bob = np.asarray(bo, np.float32).astype(bf).reshape(1, EMBED)

    wqTb = wqTb.reshape(EB, P, EMBED)
    wkTb = wkTb.reshape(EB, P, EMBED)
    wvTb = wvTb.reshape(EB, P, EMBED)
    woTb = woTb.reshape(EB, P, EMBED)

    in_maps = []
    for c in range(NCORES):
        n, qs = c // 4, (c % 4) * LQ
        xTn = np.ascontiguousarray(xb[n].T).reshape(EB, P, L)
        xqTn = np.ascontiguousarray(xb[n, qs : qs + LQ].T).reshape(EB, P, LQ)
        in_maps.append(
            {
                "xT": xTn,
                "xqT": xqTn,
                "wqT": wqTb,
                "wkT": wkTb,
                "wvT": wvTb,
                "woT": woTb,
                "bo": bob,
            }
        )
    return in_maps


def _run(x, Wq, Wk, Wv, Wo, bo, trace=False):
    nc = _get_nc()
    in_maps = _make_in_maps(x, Wq, Wk, Wv, Wo, bo)
    res = run_bass_kernel_spmd(
        nc, in_maps, core_ids=list(range(NCORES)), trace=trace
    )
    full = np.empty((N_BATCH, L, EMBED), np.float32)
    for c in range(NCORES):
        n, qs = c // 4, (c % 4) * LQ
        full[n, qs : qs + LQ] = res.results[c]["out"].reshape(LQ, EMBED)
    return full, res


def kernel(x, Wq, Wk, Wv, Wo, bo):
    full, _ = _run(x, Wq, Wk, Wv, Wo, bo, trace=False)
    return full


# revision 25
# speedup vs baseline: 1.0031x; 1.0031x over previous
"""Multi-head attention (N=2, L=2048, E=1024, H=16) on 8 TRN2 NeuronCores.

Sharding: each core owns one batch (core//4) and a 512-query slice
(core%4).  It computes K/V projections for its whole batch (replicated
4x across the cores sharing that batch), Q only for its query slice,
full softmax attention for its queries, and the output projection for
its slice.  Output shards are disjoint, so the host just concatenates —
no collectives (an on-chip 16MB AllReduce would cost ~300us, far more
than the replicated K/V matmuls).

All matmuls run in bf16 with fp32 PSUM accumulation.  The 1/sqrt(E)
score scale is folded into Wq on the host.  Softmax skips the max
subtraction (scores are ~N(0, 0.25^2) by construction — no overflow
risk) and gets the row sums for free by augmenting V with a ones
column, so the only non-matmul softmax cost is the exp itself (ACT).

Schedule: the kernel is one software pipeline over 8 head pairs.  The
K^T projection for pair j+1 and the V projection (pair 0 only) are
interleaved into pair j's score/exp/ctx stream so the PE never idles
while ACT chews through the exps, and ACT starts ~30us in instead of
after all projections.  Head pairs are stored at partition offsets
0/64 so the d=64 score matmuls of a pair run concurrently in separate
PE row groups.

Layouts on device (per core):
  xT   [e, l]   : x[n].T          — rhs for K^T, lhsT for V
  xqT  [e, q]   : x[n, qs:qs+512].T
  w*T  [e, eo]  : W.T             — lhsT for the projections
  K^T  [eo, l]  (eo = 64*h + d), Q^T [eo, q]
  V    [l, h, 65] (col 64 = ones) — lhsT for ctx^T; row 64 of the ctx
                                    PSUM then holds the softmax sums
  scores^T [k, q] -> exp -> p^T   — ctx^T[d, q] = V'.T @ p^T
  ctxN [eo, q] = ctx^T * (1/sums) — lhsT for the output projection
"""

import os
import sys
from contextlib import ExitStack

import numpy as np

if "/opt/trn_rl_repo" not in sys.path:
    sys.path.insert(0, "/opt/trn_rl_repo")

import ml_dtypes

import concourse.bass as bass
import concourse.mybir as mybir
import concourse.tile as tile
from concourse import bacc
from concourse.bass_utils import run_bass_kernel_spmd

EMBED = 1024
HEADS = 16
DHEAD = 64
N_BATCH = 2
L = 2048
LQ = 512          # queries per core
EB = 8            # 128-row blocks of the embed dim
LB = 16           # 128-row blocks of the key dim
P = 128
NCORES = 8

BF16 = mybir.dt.bfloat16
F32 = mybir.dt.float32


def _build_bass(debug=False):
    nc = bacc.Bacc()

    xT = nc.dram_tensor("xT", (EB, P, L), BF16, kind="ExternalInput")
    xqT = nc.dram_tensor("xqT", (EB, P, LQ), BF16, kind="ExternalInput")
    wqT = nc.dram_tensor("wqT", (EB, P, EMBED), BF16, kind="ExternalInput")
    wkT = nc.dram_tensor("wkT", (EB, P, EMBED), BF16, kind="ExternalInput")
    wvT = nc.dram_tensor("wvT", (EB, P, EMBED), BF16, kind="ExternalInput")
    woT = nc.dram_tensor("woT", (EB, P, EMBED), BF16, kind="ExternalInput")
    bo = nc.dram_tensor("bo", (1, EMBED), BF16, kind="ExternalInput")
    out = nc.dram_tensor("out", (LQ // P, P, EMBED), F32, kind="ExternalOutput")

    dbg = None
    if debug:
        dbg = {
            "dbg_QT": nc.dram_tensor("dbg_QT", (EB, P, LQ), BF16, kind="ExternalOutput"),
            "dbg_KT": nc.dram_tensor("dbg_KT", (EB, P, L), BF16, kind="ExternalOutput"),
            "dbg_V": nc.dram_tensor(
                "dbg_V", (LB, P, HEADS * (DHEAD + 1)), BF16, kind="ExternalOutput"
            ),
            "dbg_cN": nc.dram_tensor("dbg_cN", (EB, P, LQ), BF16, kind="ExternalOutput"),
            "dbg_ctxf": nc.dram_tensor("dbg_ctxf", (2, DHEAD + 1, LQ), F32, kind="ExternalOutput"),
            "dbg_recip": nc.dram_tensor("dbg_recip", (2, 1, LQ), F32, kind="ExternalOutput"),
            "dbg_bcs": nc.dram_tensor("dbg_bcs", (2, DHEAD, LQ), F32, kind="ExternalOutput"),
        }

    with tile.TileContext(nc) as tc, ExitStack() as ctx:
        _body(nc, tc, ctx, xT, xqT, wqT, wkT, wvT, woT, bo, out, dbg)
    nc.compile()
    return nc


def _body(nc, tc, ctx, xT, xqT, wqT, wkT, wvT, woT, bo, out, dbg=None):
    Exp = mybir.ActivationFunctionType.Exp

    persist = ctx.enter_context(tc.tile_pool(name="persist", bufs=1))

    # persistent tiles (loads for wo/bo issued later, after the hot inputs)
    wo_sb = [persist.tile([P, EMBED], BF16, tag=f"wo{i}", name=f"wo{i}") for i in range(EB)]
    bo_sb = persist.tile([1, EMBED], BF16, tag="bo", name="bo")
    ones16 = persist.tile([1, P], BF16, tag="ones16", name="ones16")
    KT_sb = [persist.tile([P, L], BF16, tag=f"KT{i}", name=f"KT{i}") for i in range(EB)]
    QT_sb = [persist.tile([P, LQ], BF16, tag=f"QT{i}", name=f"QT{i}") for i in range(EB)]
    V_sb = [
        persist.tile([P, HEADS, DHEAD + 1], BF16, tag=f"V{i}", name=f"V{i}")
        for i in range(LB)
    ]
    cN_sb = [persist.tile([P, LQ], BF16, tag=f"cN{i}", name=f"cN{i}") for i in range(EB)]

    # slab s of each 4-slab score/p tile: s0=(hA, 2g), s1=(hB, 2g),
    # s2=(hA, 2g+1), s3=(hB, 2g+1) — A/B alternate PE row groups.
    def slab(s, g):
        return s % 2, 2 * g + s // 2

    with tc.tile_pool(name="poolB", bufs=1) as poolB:
        xT_sb = [poolB.tile([P, L], BF16, tag=f"xT{i}", name=f"xT{i}") for i in range(EB)]
        wv_sb = [poolB.tile([P, EMBED], BF16, tag=f"wv{i}", name=f"wv{i}") for i in range(EB)]
        wk_sb = [poolB.tile([P, EMBED], BF16, tag=f"wk{i}", name=f"wk{i}") for i in range(EB)]

        with (
            tc.tile_pool(name="poolA", bufs=1) as poolA,
            tc.tile_pool(name="psS", bufs=1, space="PSUM") as psS,
            tc.tile_pool(name="psCtx", bufs=2, space="PSUM") as psCtx,
            tc.tile_pool(name="psV", bufs=1, space="PSUM") as psV,
            tc.tile_pool(name="ptp", bufs=2) as ptp,
            tc.tile_pool(name="smp", bufs=1) as smp,
            tc.tile_pool(name="osb", bufs=1) as osb,
        ):
            # ---- loads, hottest first ------------------------------------
            xq_sb = []
            for i in range(EB):
                t = poolA.tile([P, LQ], BF16, tag=f"xq{i}", name=f"xq{i}")
                nc.sync.dma_start(out=t, in_=xqT[i])
                xq_sb.append(t)
            wq_sb = []
            for i in range(EB):
                t = poolA.tile([P, EMBED], BF16, tag=f"wq{i}", name=f"wq{i}")
                nc.sync.dma_start(out=t, in_=wqT[i])
                wq_sb.append(t)
            for i in range(EB):
                nc.sync.dma_start(out=wk_sb[i], in_=wkT[i])
            for i in range(EB):
                nc.sync.dma_start(out=xT_sb[i], in_=xT[i])
            for i in range(EB):
                nc.sync.dma_start(out=wv_sb[i], in_=wvT[i])
            for i in range(EB):
                nc.sync.dma_start(out=wo_sb[i], in_=woT[i])
            nc.sync.dma_start(out=bo_sb, in_=bo.ap())
            nc.vector.memset(ones16, 1.0)

            # ---- prologue: all of Q^T, K^T blocks 0-1 --------------------
            for eo in range(EB):
                psq = psCtx.tile([P, LQ], F32, tag="ctx", name="psq")
                for e in range(EB):
                    nc.tensor.matmul(
                        psq,
                        wq_sb[e][:, eo * P : (eo + 1) * P],
                        xq_sb[e],
                        start=(e == 0),
                        stop=(e == EB - 1),
                    )
                nc.vector.tensor_copy(out=QT_sb[eo], in_=psq)

            def kt_half_mm(eo, half, e, psk):
                for c in range(2):
                    nc.tensor.matmul(
                        psk[:, c * 512 : (c + 1) * 512],
                        wk_sb[e][:, eo * P : (eo + 1) * P],
                        xT_sb[e][:, half * 1024 + c * 512 : half * 1024 + (c + 1) * 512],
                        start=(e == 0),
                        stop=(e == EB - 1),
                    )

            for eo in range(2):
                for half in range(2):
                    psk = psV.tile([P, 1024], F32, tag="v", name="psk")
                    for e in range(EB):
                        kt_half_mm(eo, half, e, psk)
                    nc.vector.tensor_copy(
                        out=KT_sb[eo][:, half * 1024 : (half + 1) * 1024], in_=psk
                    )

            def v_block(lb):
                psv = psV.tile([P, EMBED], F32, tag="v", name="psv")
                for e in range(EB):
                    for c in range(2):
                        nc.tensor.matmul(
                            psv[:, c * 512 : (c + 1) * 512],
                            xT_sb[e][:, lb * P : (lb + 1) * P],
                            wv_sb[e][:, c * 512 : (c + 1) * 512],
                            start=(e == 0),
                            stop=(e == EB - 1),
                        )
                nc.vector.memset(V_sb[lb][:, :, DHEAD : DHEAD + 1], 1.0)
                nc.vector.tensor_copy(
                    out=V_sb[lb][:, :, 0:DHEAD],
                    in_=psv.rearrange("p (h d) -> p h d", d=DHEAD),
                )

            # ---- pair pipeline -------------------------------------------
            for j in range(HEADS // 2):
                pts = {}
                cps = [
                    psCtx.tile([P, LQ], F32, tag="ctx", name="cpsA"),
                    psCtx.tile([P, LQ], F32, tag="ctx", name="cpsB"),
                ]
                kt_eo = j + 1  # K^T block computed during this pair (j=1..6)
                psk = None

                def ctx_group(g):
                    for s in range(4):
                        hi, kb = slab(s, g)
                        nc.tensor.matmul(
                            cps[hi][0 : DHEAD + 1, :],
                            V_sb[kb][:, 2 * j + hi, :],
                            pts[g][:, s * 512 : (s + 1) * 512],
                            start=(g == 0 and s < 2),
                            stop=(g == 7 and s >= 2),
                        )
                    if g >= 1:
                        del pts[g - 1]

                for g in range(8):
                    # scores + exp for group g
                    pss = psS.tile([P, 2048], F32, tag="s", name="pss")
                    for s in range(4):
                        hi, kb = slab(s, g)
                        off = 64 * hi
                        nc.tensor.matmul(
                            pss[:, s * 512 : (s + 1) * 512],
                            KT_sb[j][off : off + 64, kb * P : (kb + 1) * P],
                            QT_sb[j][off : off + 64, :],
                            start=True,
                            stop=True,
                        )
                    pt = ptp.tile([P, 2048], BF16, tag="pt", name="pt")
                    nc.scalar.activation(out=pt, in_=pss, func=Exp)
                    pts[g] = pt

                    if j == 0:
                        v_block(2 * g)
                        if g >= 1:
                            ctx_group(g - 1)
                        v_block(2 * g + 1)
                    else:
                        if 1 <= j <= 6:
                            half, local = g // 4, g % 4
                            if local == 0:
                                psk = psV.tile([P, 1024], F32, tag="v", name="psk")
                            for e in (2 * local, 2 * local + 1):
                                kt_half_mm(kt_eo, half, e, psk)
                            if local == 3:
                                nc.vector.tensor_copy(
                                    out=KT_sb[kt_eo][:, half * 1024 : (half + 1) * 1024],
                                    in_=psk,
                                )
                        if g >= 1:
                            ctx_group(g - 1)

                ctx_group(7)

                # normalization — everything off the PE stream
                for hi in range(2):
                    ctxf = smp.tile([DHEAD + 1, LQ], F32, tag="ctxf", name="ctxf")
                    nc.vector.tensor_copy(out=ctxf, in_=cps[hi][0 : DHEAD + 1, :])
                    recip = smp.tile([1, LQ], F32, tag="recip", name="recip")
                    nc.vector.reciprocal(out=recip, in_=ctxf[DHEAD : DHEAD + 1, :])
                    bcs = smp.tile([DHEAD, LQ], F32, tag="bcs", name="bcs")
                    nc.gpsimd.partition_broadcast(bcs, recip)
                    if dbg is not None and j == 0:
                        nc.sync.dma_start(out=dbg["dbg_ctxf"][hi], in_=ctxf)
                        nc.sync.dma_start(out=dbg["dbg_recip"][hi], in_=recip)
                        nc.sync.dma_start(out=dbg["dbg_bcs"][hi], in_=bcs)
                    nc.vector.tensor_mul(
                        cN_sb[j][64 * hi : 64 * hi + 64, :],
                        ctxf[0:DHEAD, :],
                        bcs,
                    )

            # ---- output projection + bias --------------------------------
            for qb in range(LQ // P):
                pool, tg = (psS, "s") if qb % 2 == 0 else (psV, "v")
                pso = pool.tile([P, EMBED], F32, tag=tg, name="pso")
                for eb in range(EB):
                    lhsT = cN_sb[eb][:, qb * P : (qb + 1) * P]
                    for c in range(2):
                        nc.tensor.matmul(
                            pso[:, c * 512 : (c + 1) * 512],
                            lhsT,
                            wo_sb[eb][:, c * 512 : (c + 1) * 512],
                            start=(eb == 0),
                            stop=False,
                        )
                for c in range(2):
                    nc.tensor.matmul(
                        pso[:, c * 512 : (c + 1) * 512],
                        ones16[:, 0:P],
                        bo_sb[:, c * 512 : (c + 1) * 512],
                        start=False,
                        stop=True,
                    )
                ot = osb.tile([P, EMBED], F32, tag="ot", name="ot")
                nc.vector.tensor_copy(out=ot, in_=pso)
                nc.sync.dma_start(out=out[qb], in_=ot)

            if dbg is not None:
                for i in range(EB):
                    nc.sync.dma_start(out=dbg["dbg_QT"][i], in_=QT_sb[i])
                    nc.sync.dma_start(out=dbg["dbg_KT"][i], in_=KT_sb[i])
                    nc.sync.dma_start(out=dbg["dbg_cN"][i], in_=cN_sb[i])
                for i in range(LB):
                    nc.sync.dma_start(
                        out=dbg["dbg_V"][i],
                        in_=V_sb[i].rearrange("p h d -> p (h d)"),
                    )


_NC_CACHE = None


def _get_nc():
    global _NC_CACHE
    if _NC_CACHE is None:
        _NC_CACHE = _build_bass()
    return _NC_CACHE


def _make_in_maps(x, Wq, Wk, Wv, Wo, bo):
    bf = ml_dtypes.bfloat16
    xb = np.asarray(x, dtype=np.float32).astype(bf)
    scale = 1.0 / np.sqrt(np.float32(EMBED))
    wqTb = np.ascontiguousarray(np.asarray(Wq, np.float32).T * scale).astype(bf)
    wkTb = np.ascontiguousarray(np.asarray(Wk, np.float32).T).astype(bf)
    wvTb = np.ascontiguousarray(np.asarray(Wv, np.float32).T).astype(bf)
    woTb = np.ascontiguousarray(np.asarray(Wo, np.float32).T).astype(bf)
    bob = np.asarray(bo, np.float32).astype(bf).reshape(1, EMBED)

    wqTb = wqTb.reshape(EB, P, EMBED)
    wkTb = wkTb.reshape(EB, P, EMBED)
    wvTb = wvTb.reshape(EB, P, EMBED)
    woTb = woTb.reshape(EB, P, EMBED)

    in_maps = []
    for c in range(NCORES):
        n, qs = c // 4, (c % 4) * LQ
        xTn = np.ascontiguousarray(xb[n].T).reshape(EB, P, L)
        xqTn = np.ascontiguousarray(xb[n, qs : qs + LQ].T).reshape(EB, P, LQ)
        in_maps.append(
            {
                "xT": xTn,
                "xqT": xqTn,
                "wqT": wqTb,
                "wkT": wkTb,
                "wvT": wvTb,
                "woT": woTb,
                "bo": bob,
            }
        )
    return in_maps


def _run(x, Wq, Wk, Wv, Wo, bo, trace=False):
    nc = _get_nc()
    in_maps = _make_in_maps(x, Wq, Wk, Wv, Wo, bo)
    res = run_bass_kernel_spmd(
        nc, in_maps, core_ids=list(range(NCORES)), trace=trace
    )
    full = np.empty((N_BATCH, L, EMBED), np.float32)
    for c in range(NCORES):
        n, qs = c // 4, (c % 4) * LQ
        full[n, qs : qs + LQ] = res.results[c]["out"].reshape(LQ, EMBED)
    return full, res


def kernel(x, Wq, Wk, Wv, Wo, bo):
    full, _ = _run(x, Wq, Wk, Wv, Wo, bo, trace=False)
    return full
